# revision 7
# baseline (speedup 1.0000x reference)
"""Autoformer forward on 8 trn2 NeuronCores, data-parallel over batch (4/core).

Per-batch device layouts:
  tm (time-major)   [1024 t, 512 c] : DRAM fp32 masters, SBUF staging slices [128, 512]
  cm (channel-major) [512 c, 1024 t]: SBUF fp16 flat [128, 4*1024] (slot ct)
Autocorrelation: S = x @ (wq wk^T) @ x'^T via z = Wqk^T x (cm) then S = z^T x';
mean_corr = diagonal sums of S via doubled-row DRAM write + diagonal-shear DMA
read + (1/512)*ones matmul; top-20 via max8/match_replace; masked softmax;
aggregation agg = v @ G with circulant G streamed from a broadcast Q buffer
through diagonal-shear DMAs.
"""
import sys
sys.path.insert(0, '/opt/trn_rl_repo')
import numpy as np

P = 128
L = 1024
D = 512
FF = 2048
EIN = 21
NOUT = 21
MA = 25
BLOC = 4
NCORES = 8
LABEL = 512
PRED = 512

_CACHE = {}


def _pe_table():
    pos = np.arange(L, dtype=np.float32)[:, None]
    div = np.exp(np.arange(0, D, 2, dtype=np.float32) * (-np.log(10000.0) / D))
    pe = np.zeros((L, D), np.float32)
    pe[:, 0::2] = np.sin(pos * div)
    pe[:, 1::2] = np.cos(pos * div)
    return pe


def _wma_band_flat():
    p = (MA - 1) // 2
    W = np.zeros((L, L), np.float64)
    for t in range(L):
        for u in range(t - p, t + p + 1):
            s = min(max(u, 0), L - 1)
            W[t, s] += 1.0 / MA
    out = np.zeros((P, 24 * P), np.float16)
    for mt in range(8):
        for k in range(3):
            st = mt - 1 + k
            if st < 0 or st > 7:
                continue
            blk = W[mt * P:(mt + 1) * P, st * P:(st + 1) * P].T
            out[:, (mt * 3 + k) * P:(mt * 3 + k + 1) * P] = blk.astype(np.float16)
    return out


def _f16(x):
    return np.ascontiguousarray(np.asarray(x, np.float32).astype(np.float16))


def _flat_params(params):
    t = {}
    t['pe_tm'] = _pe_table()
    t['wma_band'] = _wma_band_flat()
    for nm, w in (('etok', params['enc_token_w']), ('dtok', params['dec_token_w'])):
        w = np.asarray(w, np.float32)
        t[nm] = _f16(np.stack([w[:, :, j].T for j in range(3)]))
    attns = [('e0', params['enc_layers'][0]['attn']),
             ('e1', params['enc_layers'][1]['attn'])]
    dp = params['dec_layers'][0]
    attns += [('ds', dp['self']), ('dc', dp['cross'])]
    for nm, a in attns:
        wqk = np.asarray(a['wq'], np.float64) @ np.asarray(a['wk'], np.float64).T
        hi = wqk.astype(np.float16)
        lo = (wqk - hi.astype(np.float64)).astype(np.float16)
        t[f'{nm}_qk_hi'] = np.ascontiguousarray(hi)
        t[f'{nm}_qk_lo'] = np.ascontiguousarray(lo)
        t[f'{nm}_wv'] = _f16(a['wv'])
        t[f'{nm}_wo'] = _f16(a['wo'])
    for li in range(2):
        t[f'e{li}_w1'] = _f16(params['enc_layers'][li]['w1'])
        t[f'e{li}_w2'] = _f16(params['enc_layers'][li]['w2'])
    t['d_w1'] = _f16(dp['w1'])
    t['d_w2'] = _f16(dp['w2'])
    wt = np.asarray(dp['wtrend'], np.float32)
    t['wtrend'] = _f16(np.stack([wt[:, :, j].T for j in range(3)]))
    t['proj_w'] = _f16(params['proj_w'])
    t['trend_proj'] = _f16(params['trend_proj_w'])
    t['ident'] = np.eye(P, dtype=np.float16)
    return t


DEBUG = False


def _build_program():
    import concourse.bass as bass
    import concourse.bacc as bacc
    import concourse.mybir as mybir
    from concourse.tile import TileContext

    FP16 = mybir.dt.float16
    FP32 = mybir.dt.float32
    AX = mybir.AxisListType.X
    ALU = mybir.AluOpType
    ACTF = mybir.ActivationFunctionType

    nc = bacc.Bacc("TRN2", target_bir_lowering=False, debug=False,
                   enable_asserts=True, num_devices=1)

    di = {}

    def inp(name, shape, dt=FP16):
        di[name] = nc.dram_tensor(name, list(shape), dt, kind="ExternalInput")

    inp('x_tm32', (BLOC, L, EIN), FP32)
    inp('x_tm16', (BLOC, L, EIN), FP16)
    inp('x_cm16', (BLOC, EIN, L + 2), FP16)
    inp('pe_tm', (L, D), FP32)
    inp('wma_band', (P, 24 * P))
    inp('etok', (3, EIN, D)); inp('dtok', (3, EIN, D))
    for nm in ('e0', 'e1', 'ds', 'dc'):
        inp(f'{nm}_qk_hi', (D, D)); inp(f'{nm}_qk_lo', (D, D))
        inp(f'{nm}_wv', (D, D)); inp(f'{nm}_wo', (D, D))
    for nm in ('e0', 'e1', 'd'):
        inp(f'{nm}_w1', (D, FF)); inp(f'{nm}_w2', (FF, D))
    inp('wtrend', (3, D, NOUT))
    inp('proj_w', (D, NOUT))
    inp('trend_proj', (NOUT, NOUT))
    inp('ident', (P, P))

    OUTT = nc.dram_tensor('out', [BLOC, NOUT, PRED], FP32, kind="ExternalOutput")
    DBG = {}
    if DEBUG:
        for nm, shp, dt in (('dbg_x0', (L, D), FP32), ('dbg_mc', (1, L), FP32),
                            ('dbg_tp', (1, L), FP32), ('dbg_x1', (L, D), FP32),
                            ('dbg_x2', (L, D), FP32), ('dbg_x3', (L, D), FP32),
                            ('dbg_xe', (L, D), FP32), ('dbg_enc', (P, 4 * L), FP16),
                            ('dbg_tri', (NOUT, L), FP32), ('dbg_dx0', (L, D), FP32),
                            ('dbg_dx1', (L, D), FP32), ('dbg_dx7', (L, D), FP32),
                            ('dbg_tsum', (P, 8 * D), FP32), ('dbg_agg', (P, 8 * 512), FP16)):
            DBG[nm] = nc.dram_tensor(nm, list(shp), dt, kind="ExternalOutput")
    SSc = nc.dram_tensor('SSc', [2, L, 2 * L], FP16, kind="Internal")
    QB = nc.dram_tensor('QB', [2, P, 2 * L], FP16, kind="Internal")
    ENC = nc.dram_tensor('ENCDR', [BLOC, P, 4 * L], FP16, kind="Internal")
    TMEAN = nc.dram_tensor('TMEANDR', [2, D], FP32, kind="Internal")
    TREND = nc.dram_tensor('TRENDDR', [BLOC, NOUT, L], FP32, kind="Internal")
    DSEA = nc.dram_tensor('DSEADR', [BLOC, EIN, L + 2], FP16, kind="Internal")

    with TileContext(nc) as tc:
        _body(nc, tc, di, OUTT, SSc, QB, ENC, TMEAN, TREND, DSEA,
              bass, FP16, FP32, AX, ALU, ACTF, DBG)
    nc.compile()
    from concourse.bass_interp import get_hw_module
    nc.m = get_hw_module(nc.m)
    return nc


def _body(nc, tc, di, OUTT, SSc, QB, ENC, TMEAN, TREND, DSEA,
          bass, FP16, FP32, AX, ALU, ACTF, DBG):
    def dbg_dump(nm, src_ap):
        if nm in DBG:
            nc.sync.dma_start(DBG[nm][:, :], src_ap)
    from contextlib import ExitStack
    ctx = ExitStack()

    def pool(name, bufs, space="SBUF"):
        return ctx.enter_context(tc.tile_pool(name=name, bufs=bufs, space=space))

    const = pool("const", 1)
    wpool = pool("wpool", 1)
    wff = pool("wff", 1)
    stg = pool("stg", 4)       # st32 / ld32 / c16 staging slices
    cma = pool("cma", 2)       # cm fp16 flats
    hpool = pool("hp", 1)      # ffn hidden (t-half)
    med = pool("med", 2)       # zhi/v/agg/aggcm
    gbp = pool("gbp", 8)       # G blocks
    encp = pool("encp", 1)
    scp = pool("scp", 2)
    tkp = pool("tkp", 2)
    dmast = pool("dmast", 6, "DRAM")
    psmm = pool("psmm", 3, "PSUM")
    psx = pool("psx", 1, "PSUM")

    # ---------------- consts ----------------
    ident = const.tile([P, P], FP16)
    nc.sync.dma_start(ident[:], di['ident'][:, :])
    ones512 = const.tile([P, 1], FP16)
    nc.vector.memset(ones512[:], 1.0 / 512)
    onesLN = const.tile([P, 1], FP16)
    nc.vector.memset(onesLN[:], 1.0 / 1024)
    wma = const.tile([P, 24 * P], FP16)
    nc.sync.dma_start(wma[:], di['wma_band'][:, :])
    etok = const.tile([EIN, 3 * D], FP16, name="etok")
    dtok = const.tile([EIN, 3 * D], FP16, name="dtok")
    for j in range(3):
        nc.sync.dma_start(etok[:, j * D:(j + 1) * D], di['etok'][j, :, :])
        nc.sync.dma_start(dtok[:, j * D:(j + 1) * D], di['dtok'][j, :, :])
    wtr = const.tile([P, 12 * NOUT], FP16, name="wtr")
    for j in range(3):
        for ct in range(4):
            nc.sync.dma_start(wtr[:, (j * 4 + ct) * NOUT:(j * 4 + ct + 1) * NOUT],
                              di['wtrend'][j, ct * P:(ct + 1) * P, :])
    projw = const.tile([P, 4 * NOUT], FP16, name="projw")
    for ct in range(4):
        nc.sync.dma_start(projw[:, ct * NOUT:(ct + 1) * NOUT],
                          di['proj_w'][ct * P:(ct + 1) * P, :])
    tprojw = const.tile([NOUT, NOUT], FP16, name="tprojw")
    nc.sync.dma_start(tprojw[:], di['trend_proj'][:, :])

    uid = [0]

    def un(s):
        uid[0] += 1
        return f"{s}_{uid[0]}"

    def new_master():
        return dmast.tile([L, D], FP32, tag="mast", name=un("mast"))

    def ld32(m, mt):
        t = stg.tile([P, D], FP32, tag="ld32", bufs=3, name=un("ld"))
        nc.sync.dma_start(t[:], m[mt * P:(mt + 1) * P, :])
        return t

    def c16_of(t32):
        t = stg.tile([P, D], FP16, tag="c16", bufs=6, name=un("c16"))
        nc.scalar.copy(t[:], t32[:])
        return t

    def trans4(src16, mt, cm_dst, cm_w=L):
        # src16 [128 t, 512 c] slice at t-tile mt -> 4 transposes into cm_dst
        for ci in range(4):
            pt = psx.tile([P, P], FP16, tag="tp", bufs=2, name=un("tp"))
            nc.tensor.matmul(pt[:], src16[:, ci * P:(ci + 1) * P], ident[:],
                             is_transpose=True, start=True, stop=True)
            nc.scalar.copy(cm_dst[:, ci * cm_w + mt * P: ci * cm_w + (mt + 1) * P], pt[:])

    def evict(pd, mt, out_m, resid_m=None, sign=1.0, cm_dst=None, tsum=None,
              need16=False):
        """st = sign*pd + resid; DMA to out_m; optionally cast+transpose to cm_dst,
        optionally tsum += pd. Returns fp16 slice if need16."""
        st = stg.tile([P, D], FP32, tag="st32", bufs=3, name=un("st"))
        if resid_m is not None:
            rl = ld32(resid_m, mt)
            nc.vector.scalar_tensor_tensor(out=st[:], in0=pd[:], scalar=sign,
                                           in1=rl[:], op0=ALU.mult, op1=ALU.add)
        else:
            nc.scalar.copy(st[:], pd[:])
        nc.sync.dma_start(out_m[mt * P:(mt + 1) * P, :], st[:])
        if tsum is not None:
            nc.vector.scalar_tensor_tensor(
                out=tsum[:, mt * D:(mt + 1) * D], in0=pd[:], scalar=1.0,
                in1=tsum[:, mt * D:(mt + 1) * D], op0=ALU.mult, op1=ALU.add)
        t16 = None
        if cm_dst is not None or need16:
            t16 = c16_of(st)
            if cm_dst is not None:
                trans4(t16, mt, cm_dst)
        return t16

    def load_w(dram, nkt, width, pool_, tag, bufs=None):
        # dram [nkt*128, width] -> SBUF [128, nkt*width], slot kt
        t = pool_.tile([P, nkt * width], FP16, tag=tag, name=un(tag),
                       **({} if bufs is None else {'bufs': bufs}))
        for kt in range(nkt):
            nc.sync.dma_start(t[:, kt * width:(kt + 1) * width],
                              dram[kt * P:(kt + 1) * P, :])
        return t

    ssc_i = [0]

    # ---------------- attention ----------------
    def attention(b, nm, xq_m, xqcm, kvcm):
        qk_hi = load_w(di[f'{nm}_qk_hi'], 4, D, wpool, "qkhi")
        qk_lo = load_w(di[f'{nm}_qk_lo'], 4, D, wpool, "qklo")
        wv = load_w(di[f'{nm}_wv'], 4, D, wpool, "wv")
        wo = load_w(di[f'{nm}_wo'], 4, D, wpool, "wo")
        slot = ssc_i[0] % 2
        ssc_i[0] += 1

        zhi = med.tile([P, 4 * L], FP16, tag="med", name=un("zhi"))
        for ntt in range(4):
            for tch in range(2):
                pz = psmm.tile([P, 512], FP32, tag="mm", name=un("pz"))
                for ct in range(4):
                    nc.tensor.matmul(pz[:], qk_hi[:, ct * D + ntt * P: ct * D + (ntt + 1) * P],
                                     xqcm[:, ct * L + tch * 512: ct * L + (tch + 1) * 512],
                                     start=(ct == 0), stop=False)
                for ct in range(4):
                    nc.tensor.matmul(pz[:], qk_lo[:, ct * D + ntt * P: ct * D + (ntt + 1) * P],
                                     xqcm[:, ct * L + tch * 512: ct * L + (tch + 1) * 512],
                                     start=False, stop=(ct == 3))
                nc.scalar.copy(zhi[:, ntt * L + tch * 512: ntt * L + (tch + 1) * 512], pz[:])
        for rt in range(8):
            for tch in range(2):
                ps = psmm.tile([P, 512], FP32, tag="mm", name=un("ps"))
                for ct in range(4):
                    nc.tensor.matmul(ps[:], zhi[:, ct * L + rt * P: ct * L + (rt + 1) * P],
                                     kvcm[:, ct * L + tch * 512: ct * L + (tch + 1) * 512],
                                     start=(ct == 0), stop=(ct == 3))
                s16t = scp.tile([P, 512], FP16, tag="s16t", name=un("s16"))
                nc.scalar.copy(s16t[:], ps[:])
                nc.sync.dma_start(SSc[slot, rt * P:(rt + 1) * P, tch * 512:(tch + 1) * 512], s16t[:])
                nc.sync.dma_start(SSc[slot, rt * P:(rt + 1) * P, L + tch * 512:L + (tch + 1) * 512], s16t[:])
        mc0 = psx.tile([1, 512], FP32, tag="mc0", bufs=1, name=un("mc0"))
        mc1 = psx.tile([1, 512], FP32, tag="mc1", bufs=1, name=un("mc1"))
        for blk in range(8):
            t16 = scp.tile([P, L], FP16, tag="shear", bufs=1, name=un("sh"))
            src = bass.AP(tensor=SSc, offset=slot * L * 2 * L + P * blk * (2 * L + 1),
                          ap=[[2 * L + 1, P], [1, L]])
            nc.sync.dma_start(t16[:], src)
            nc.tensor.matmul(mc0[:], ones512[:], t16[:, 0:512], start=(blk == 0), stop=(blk == 7))
            nc.tensor.matmul(mc1[:], ones512[:], t16[:, 512:L], start=(blk == 0), stop=(blk == 7))
        mc = tkp.tile([1, L], FP32, tag="tkbig", bufs=3, name=un("mc"))
        nc.scalar.copy(mc[:, 0:512], mc0[:])
        nc.scalar.copy(mc[:, 512:L], mc1[:])
        if b == 0 and nm == 'e0':
            dbg_dump('dbg_mc', mc[:])
        mn = tkp.tile([1, 1], FP32, tag="mn", name=un("mn"))
        nc.vector.tensor_reduce(out=mn[:], in_=mc[:], op=ALU.min, axis=AX)
        mcp = tkp.tile([1, L], FP32, tag="tkbig", bufs=3, name=un("mcp"))
        nc.vector.tensor_scalar(out=mcp[:], in0=mc[:], scalar1=mn[:], scalar2=1.0,
                                op0=ALU.subtract, op1=ALU.add)
        work = tkp.tile([1, L], FP32, tag="tkbig", bufs=3, name=un("wk"))
        nc.vector.tensor_copy(work[:], mcp[:])
        mx8 = tkp.tile([1, 8], FP32, tag="mx8", name=un("mx8"))
        for kk in (8, 8, 4):
            nc.vector.max(mx8[:], work[:])
            if kk < 8:
                nc.vector.memset(mx8[:, kk:8], 0.0)
            nc.vector.match_replace(out=work[:], in_to_replace=mx8[:], in_values=work[:],
                                    imm_value=0.0)
        mask = tkp.tile([1, L], FP32, tag="tkbig", bufs=3, name=un("msk"))
        nc.vector.tensor_sub(mask[:], mcp[:], work[:])
        nc.vector.tensor_scalar(out=mask[:], in0=mask[:], scalar1=0.0, scalar2=None,
                                op0=ALU.is_gt)
        mxv = tkp.tile([1, 1], FP32, tag="mxv", name=un("mxv"))
        nc.vector.tensor_reduce(out=mxv[:], in_=mcp[:], op=ALU.max, axis=AX)
        nc.vector.tensor_scalar_mul(mxv[:], mxv[:], -1.0)
        eall = tkp.tile([1, L], FP32, tag="tkbig", bufs=3, name=un("ea"))
        nc.scalar.activation(eall[:], mcp[:], ACTF.Exp, bias=mxv[:], scale=1.0)
        nc.vector.tensor_mul(eall[:], eall[:], mask[:])
        den = tkp.tile([1, 1], FP32, tag="den", name=un("dn"))
        nc.vector.tensor_reduce(out=den[:], in_=eall[:], op=ALU.add, axis=AX)
        nc.vector.reciprocal(den[:], den[:])
        nc.vector.tensor_scalar_mul(eall[:], eall[:], den[:])
        tp16 = tkp.tile([1, L], FP16, tag="tp16", bufs=1, name=un("tp"))
        nc.vector.tensor_copy(tp16[:], eall[:])
        if b == 0 and nm == 'e0':
            dbg_dump('dbg_tp', eall[:])
        nc.sync.dma_start(QB[slot, 0:1, 0:L], tp16[:])
        nc.sync.dma_start(QB[slot, 0:1, L:2 * L], tp16[:])
        tbc = scp.tile([P, 2 * L], FP16, tag="tbc", bufs=1, name=un("tbc"))
        qsrc = bass.AP(tensor=QB, offset=slot * P * 2 * L, ap=[[0, P], [1, 2 * L]])
        nc.gpsimd.dma_start(out=tbc[:], in_=qsrc)
        nc.sync.dma_start(QB[slot, :, :], tbc[:])
        # v (tm fp16)
        v16 = med.tile([P, 8 * 512], FP16, tag="med", name=un("v"))
        for mt in range(8):
            pv = psmm.tile([P, 512], FP32, tag="mm", name=un("pv"))
            for ct in range(4):
                nc.tensor.matmul(pv[:], kvcm[:, ct * L + mt * P: ct * L + (mt + 1) * P],
                                 wv[:, ct * D:(ct + 1) * D],
                                 start=(ct == 0), stop=(ct == 3))
            nc.scalar.copy(v16[:, mt * 512:(mt + 1) * 512], pv[:])
        # agg = sum_m G[m, l] v[m, c], G blocks streamed from QB
        agg = med.tile([P, 8 * 512], FP16, tag="med", name=un("agg"))
        for lt in range(8):
            pa = psmm.tile([P, 512], FP32, tag="mm", name=un("pa"))
            for mt in range(8):
                gb = gbp.tile([P, P], FP16, tag="gb", name=un("gb"))
                gsrc = bass.AP(tensor=QB,
                               offset=slot * P * 2 * L + (L - P * mt) + lt * P,
                               ap=[[2 * L - 1, P], [1, P]])
                nc.sync.dma_start(gb[:], gsrc)
                nc.tensor.matmul(pa[:], gb[:], v16[:, mt * 512:(mt + 1) * 512],
                                 start=(mt == 0), stop=(mt == 7))
            nc.scalar.copy(agg[:, lt * 512:(lt + 1) * 512], pa[:])
        if b == 0 and nm == 'e0':
            dbg_dump('dbg_agg', agg[:])
        # transpose agg -> cm
        aggcm = med.tile([P, 4 * L], FP16, tag="med", name=un("agc"))
        for lt in range(8):
            trans4(agg[:, lt * 512:(lt + 1) * 512], lt, aggcm)
        # o proj + residual
        x1_m = new_master()
        for mt in range(8):
            po = psmm.tile([P, 512], FP32, tag="mm", name=un("po"))
            for ct in range(4):
                nc.tensor.matmul(po[:], aggcm[:, ct * L + mt * P: ct * L + (mt + 1) * P],
                                 wo[:, ct * D:(ct + 1) * D],
                                 start=(ct == 0), stop=(ct == 3))
            evict(po, mt, x1_m, resid_m=xq_m)
        return x1_m

    def decomp(b, xin_m, tsum=None, cm_dst=None):
        xo_m = new_master()
        c16cache = {}
        for mt in range(8):
            pd = psmm.tile([P, 512], FP32, tag="mm", name=un("pd"))
            ks = [k for k in range(3) if 0 <= mt - 1 + k <= 7]
            for i, k in enumerate(ks):
                st = mt - 1 + k
                if st not in c16cache:
                    c16cache[st] = c16_of(ld32(xin_m, st))
                nc.tensor.matmul(pd[:], wma[:, (mt * 3 + k) * P:(mt * 3 + k + 1) * P],
                                 c16cache[st][:],
                                 start=(i == 0), stop=(i == len(ks) - 1))
            c16cache = {k_: v_ for k_, v_ in c16cache.items() if k_ >= mt}
            evict(pd, mt, xo_m, resid_m=xin_m, sign=-1.0, cm_dst=cm_dst, tsum=tsum)
        return xo_m

    def ffn(b, nm, xin_m, xincm):
        w2 = load_w(di[f'{nm}_w2'], 16, D, wff, "w2")
        xo_m = new_master()
        HW = FF // 2
        for tch in range(2):
            h = hpool.tile([P, 16 * 512], FP16, tag="h", name=un("h"))
            for half in range(2):
                w1h = wff.tile([P, 4 * HW], FP16, tag="w1h", bufs=1, name=un("w1h"))
                for ct in range(4):
                    nc.sync.dma_start(w1h[:, ct * HW:(ct + 1) * HW],
                                      di[f'{nm}_w1'][ct * P:(ct + 1) * P, half * HW:(half + 1) * HW])
                for ftl in range(8):
                    ft = half * 8 + ftl
                    ph = psmm.tile([P, 512], FP32, tag="mm", name=un("ph"))
                    for ct in range(4):
                        nc.tensor.matmul(ph[:], w1h[:, ct * HW + ftl * P: ct * HW + (ftl + 1) * P],
                                         xincm[:, ct * L + tch * 512: ct * L + (tch + 1) * 512],
                                         start=(ct == 0), stop=(ct == 3))
                    nc.scalar.activation(h[:, ft * 512:(ft + 1) * 512], ph[:], ACTF.Gelu)
            for mtl in range(4):
                mt = tch * 4 + mtl
                py = psmm.tile([P, 512], FP32, tag="mm", name=un("py"))
                for ft in range(16):
                    nc.tensor.matmul(py[:], h[:, ft * 512 + mtl * P: ft * 512 + (mtl + 1) * P],
                                     w2[:, ft * D:(ft + 1) * D],
                                     start=(ft == 0), stop=(ft == 15))
                evict(py, mt, xo_m, resid_m=xin_m)
        return xo_m

    def layernorm(b, xin_m, slot, out_cm, out_dram=None):
        xh16s = []
        for mt in range(8):
            xs = ld32(xin_m, mt)
            mu = tkp.tile([P, 1], FP32, tag="mu", name=un("mu"))
            nc.vector.tensor_reduce(out=mu[:], in_=xs[:], op=ALU.add, axis=AX)
            nc.vector.tensor_scalar_mul(mu[:], mu[:], 1.0 / D)
            sq = scp.tile([P, D], FP16, tag="sq", name=un("sq"))
            e2 = tkp.tile([P, 1], FP32, tag="e2", name=un("e2"))
            nc.scalar.activation(sq[:], xs[:], ACTF.Square, accum_out=e2[:])
            nc.vector.tensor_scalar_mul(e2[:], e2[:], 1.0 / D)
            mu2 = tkp.tile([P, 1], FP32, tag="mu2", name=un("mu2"))
            nc.vector.tensor_mul(mu2[:], mu[:], mu[:])
            nc.vector.tensor_sub(e2[:], e2[:], mu2[:])
            nc.vector.tensor_scalar_add(e2[:], e2[:], 1e-5)
            nc.scalar.activation(e2[:], e2[:], ACTF.Sqrt)
            nc.vector.reciprocal(e2[:], e2[:])
            xh = stg.tile([P, D], FP16, tag="lnx", bufs=8, name=un("xh"))
            nc.vector.tensor_scalar(out=xh[:], in0=xs[:], scalar1=mu[:], scalar2=e2[:],
                                    op0=ALU.subtract, op1=ALU.mult)
            xh16s.append(xh)
        tmp = psx.tile([1, 512], FP32, tag="mc0", bufs=1, name=un("tmps"))
        for mt in range(8):
            nc.tensor.matmul(tmp[:], onesLN[:], xh16s[mt][:], start=(mt == 0), stop=(mt == 7))
        tmsb = tkp.tile([1, D], FP32, tag="tmsb", bufs=1, name=un("tms"))
        nc.scalar.copy(tmsb[:], tmp[:])
        nc.sync.dma_start(TMEAN[slot:slot + 1, :], tmsb[:])
        tmcm = tkp.tile([P, 4], FP32, tag="tmcm", name=un("tmc"))
        tsrc = bass.AP(tensor=TMEAN, offset=slot * D, ap=[[1, P], [P, 4]])
        nc.sync.dma_start(tmcm[:], tsrc)
        for mt in range(8):
            for ci in range(4):
                pt = psx.tile([P, P], FP16, tag="tp", bufs=2, name=un("tpl"))
                nc.tensor.matmul(pt[:], xh16s[mt][:, ci * P:(ci + 1) * P], ident[:],
                                 is_transpose=True, start=True, stop=True)
                nc.vector.tensor_scalar(out=out_cm[:, ci * L + mt * P: ci * L + (mt + 1) * P],
                                        in0=pt[:], scalar1=tmcm[:, ci:ci + 1], scalar2=None,
                                        op0=ALU.subtract)
        if out_dram is not None:
            nc.sync.dma_start(out_dram[:, :], out_cm[:])

    # NOTE: xh16s holds 8 c16-tagged tiles simultaneously -> needs bufs >= 9 on
    # that tag during LN; handled by dedicated tag below instead.

    # ================= phase A: encoder per batch =================
    for b in range(BLOC):
        xcmw = scp.tile([EIN, L + 2], FP16, tag="xcmw", bufs=1, name=un("xcmw"))
        nc.sync.dma_start(xcmw[:], di['x_cm16'][b, :, :])
        xtm16b = scp.tile([P, 8 * EIN], FP16, tag="xtm16b", name=un("xtm16b"))
        xtm32b = scp.tile([P, 8 * EIN], FP32, tag="xtm32b", name=un("xtm32b"))
        for mt in range(8):
            nc.sync.dma_start(xtm16b[:, mt * EIN:(mt + 1) * EIN],
                              di['x_tm16'][b, mt * P:(mt + 1) * P, :])
            nc.sync.dma_start(xtm32b[:, mt * EIN:(mt + 1) * EIN],
                              di['x_tm32'][b, mt * P:(mt + 1) * P, :])
        # init decomp + trend init + dec seasonal input
        trinit = scp.tile([NOUT, L], FP32, tag="trinit", bufs=1, name=un("tri"))
        dsea = scp.tile([EIN, L + 2], FP16, tag="dsea", bufs=1, name=un("dsea"))
        nc.vector.memset(dsea[:], 0.0)
        for mt in range(4, 8):
            pd = psmm.tile([P, EIN], FP32, tag="mm", name=un("pini"))
            ks = [k for k in range(3) if 0 <= mt - 1 + k <= 7]
            for i, k in enumerate(ks):
                st = mt - 1 + k
                nc.tensor.matmul(pd[:], wma[:, (mt * 3 + k) * P:(mt * 3 + k + 1) * P],
                                 xtm16b[:, st * EIN:(st + 1) * EIN],
                                 start=(i == 0), stop=(i == len(ks) - 1))
            if mt >= 4:
                seas16 = scp.tile([P, EIN], FP16, tag="seas16", name=un("se16"))
                nc.vector.scalar_tensor_tensor(
                    out=seas16[:], in0=pd[:], scalar=-1.0,
                    in1=xtm32b[:, mt * EIN:(mt + 1) * EIN], op0=ALU.mult, op1=ALU.add)
                pds = psx.tile([EIN, P], FP16, tag="tp", bufs=2, name=un("pds"))
                nc.tensor.matmul(pds[:], seas16[:], ident[:], is_transpose=True,
                                 start=True, stop=True)
                nc.scalar.copy(dsea[:, 1 + (mt - 4) * P:1 + (mt - 3) * P], pds[:])
                mm16 = scp.tile([P, EIN], FP16, tag="mm16", name=un("mm16"))
                nc.scalar.copy(mm16[:], pd[:])
                ptr = psx.tile([EIN, P], FP16, tag="tp", bufs=2, name=un("ptr"))
                nc.tensor.matmul(ptr[:], mm16[:], ident[:], is_transpose=True,
                                 start=True, stop=True)
                nc.scalar.copy(trinit[:, (mt - 4) * P:(mt - 3) * P], ptr[:])
        nc.vector.tensor_copy(dsea[:, L + 1:L + 2], dsea[:, 1:2])
        meanc = tkp.tile([EIN, 1], FP32, tag="meanc", name=un("meanc"))
        nc.vector.tensor_reduce(out=meanc[:], in_=xcmw[:, 1:L + 1], op=ALU.add, axis=AX)
        nc.vector.tensor_scalar_mul(meanc[:], meanc[:], 1.0 / L)
        nc.vector.memset(trinit[:, LABEL:L], 0.0)
        nc.vector.tensor_scalar_add(trinit[:, LABEL:L], trinit[:, LABEL:L], meanc[:])
        nc.sync.dma_start(TREND[b, :, :], trinit[:])
        nc.sync.dma_start(DSEA[b, :, :], dsea[:])

        # enc embedding -> master + cm
        x_m = new_master()
        xcm = cma.tile([P, 4 * L], FP16, tag="cma", name=un("xcm"))
        for mt in range(8):
            pemb = psmm.tile([P, D], FP32, tag="mm", name=un("pemb"))
            for j in range(3):
                nc.tensor.matmul(pemb[:], xcmw[:, j + mt * P: j + mt * P + P],
                                 etok[:, j * D:(j + 1) * D],
                                 start=(j == 0), stop=(j == 2))
            pet = stg.tile([P, D], FP32, tag="ld32", bufs=3, name=un("pet"))
            nc.sync.dma_start(pet[:], di['pe_tm'][mt * P:(mt + 1) * P, :])
            st = stg.tile([P, D], FP32, tag="st32", bufs=3, name=un("stemb"))
            nc.vector.tensor_add(st[:], pemb[:], pet[:])
            nc.sync.dma_start(x_m[mt * P:(mt + 1) * P, :], st[:])
            trans4(c16_of(st), mt, xcm)

        if b == 0:
            dbg_dump('dbg_x0', x_m[:, :])
        # 2 encoder layers
        for li in range(2):
            nm = f'e{li}'
            x1_m = attention(b, nm, x_m, xcm, xcm)
            if b == 0 and li == 0:
                dbg_dump('dbg_x1', x1_m[:, :])
            xcm_f = cma.tile([P, 4 * L], FP16, tag="cma", name=un("xcmf"))
            x2_m = decomp(b, x1_m, cm_dst=xcm_f)
            if b == 0 and li == 0:
                dbg_dump('dbg_x2', x2_m[:, :])
            x3_m = ffn(b, nm, x2_m, xcm_f)
            if b == 0 and li == 0:
                dbg_dump('dbg_x3', x3_m[:, :])
            if li == 0:
                xcm = cma.tile([P, 4 * L], FP16, tag="cma", name=un("xcm1"))
                x_m = decomp(b, x3_m, cm_dst=xcm)
            else:
                x_m = decomp(b, x3_m)
        if b == 0:
            dbg_dump('dbg_xe', x_m[:, :])
        enccm = encp.tile([P, 4 * L], FP16, tag="enccm", name=un("elncm"))
        layernorm(b, x_m, 0, enccm, out_dram=ENC[b])
        if b == 0:
            dbg_dump('dbg_enc', enccm[:])
            dbg_dump('dbg_tri', trinit[:])

    # ================= phase B: decoder per batch =================
    for b in range(BLOC):
        dsea = scp.tile([EIN, L + 2], FP16, tag="dsea", bufs=1, name=un("dseal"))
        nc.sync.dma_start(dsea[:], DSEA[b, :, :])
        x_m = new_master()
        dcm = cma.tile([P, 4 * L], FP16, tag="cma", name=un("dcm"))
        for mt in range(8):
            pemb = psmm.tile([P, D], FP32, tag="mm", name=un("pembd"))
            for j in range(3):
                nc.tensor.matmul(pemb[:], dsea[:, j + mt * P: j + mt * P + P],
                                 dtok[:, j * D:(j + 1) * D],
                                 start=(j == 0), stop=(j == 2))
            pet = stg.tile([P, D], FP32, tag="ld32", bufs=3, name=un("petd"))
            nc.sync.dma_start(pet[:], di['pe_tm'][mt * P:(mt + 1) * P, :])
            st = stg.tile([P, D], FP32, tag="st32", bufs=3, name=un("stembd"))
            nc.vector.tensor_add(st[:], pemb[:], pet[:])
            nc.sync.dma_start(x_m[mt * P:(mt + 1) * P, :], st[:])
            trans4(c16_of(st), mt, dcm)

        if b == 0:
            dbg_dump('dbg_dx0', x_m[:, :])
        tsum = scp.tile([P, 8 * D], FP32, tag="tsum", bufs=1, name=un("tsum"))
        nc.vector.memset(tsum[:], 0.0)
        x1_m = attention(b, 'ds', x_m, dcm, dcm)
        if b == 0:
            dbg_dump('dbg_dx1', x1_m[:, :])
        x2cm = cma.tile([P, 4 * L], FP16, tag="cma", name=un("x2cm"))
        x2_m = decomp(b, x1_m, tsum=tsum, cm_dst=x2cm)
        enccm = encp.tile([P, 4 * L], FP16, tag="enccm", name=un("encld"))
        nc.sync.dma_start(enccm[:], ENC[b][:, :])
        x4_m = attention(b, 'dc', x2_m, x2cm, enccm)
        xcm_f = cma.tile([P, 4 * L], FP16, tag="cma", name=un("xcmfd"))
        x5_m = decomp(b, x4_m, tsum=tsum, cm_dst=xcm_f)
        x6_m = ffn(b, 'd', x5_m, xcm_f)
        x7_m = decomp(b, x6_m, tsum=tsum)
        if b == 0:
            dbg_dump('dbg_dx7', x7_m[:, :])
            dbg_dump('dbg_tsum', tsum[:])

        # trend conv
        tscm = cma.tile([P, 4 * (L + 2)], FP16, tag="cma", name=un("tscm"))
        for mt in range(8):
            t16 = stg.tile([P, D], FP16, tag="c16", bufs=6, name=un("ts16"))
            nc.scalar.copy(t16[:], tsum[:, mt * D:(mt + 1) * D])
            for ci in range(4):
                pt = psx.tile([P, P], FP16, tag="tp", bufs=2, name=un("tpt"))
                nc.tensor.matmul(pt[:], t16[:, ci * P:(ci + 1) * P], ident[:],
                                 is_transpose=True, start=True, stop=True)
                nc.scalar.copy(tscm[:, ci * (L + 2) + 1 + mt * P: ci * (L + 2) + 1 + (mt + 1) * P],
                               pt[:])
        for ci in range(4):
            base = ci * (L + 2)
            nc.vector.tensor_copy(tscm[:, base:base + 1], tscm[:, base + L:base + L + 1])
            nc.vector.tensor_copy(tscm[:, base + L + 1:base + L + 2], tscm[:, base + 1:base + 2])
        trsb = scp.tile([NOUT, L], FP32, tag="trinit", bufs=1, name=un("trsb"))
        nc.sync.dma_start(trsb[:], TREND[b, :, :])
        for tch in range(2):
            prt = psmm.tile([NOUT, 512], FP32, tag="mm", name=un("prt"))
            idx = 0
            for j in range(3):
                for ct in range(4):
                    nc.tensor.matmul(prt[:],
                                     wtr[:, (j * 4 + ct) * NOUT:(j * 4 + ct + 1) * NOUT],
                                     tscm[:, ct * (L + 2) + j + tch * 512: ct * (L + 2) + j + tch * 512 + 512],
                                     start=(idx == 0), stop=(idx == 11))
                    idx += 1
            nc.vector.scalar_tensor_tensor(
                out=trsb[:, tch * 512:(tch + 1) * 512], in0=prt[:], scalar=1.0,
                in1=trsb[:, tch * 512:(tch + 1) * 512], op0=ALU.mult, op1=ALU.add)

        # final LN + projections (last 512 t only)
        deccm = encp.tile([P, 4 * L], FP16, tag="enccm", name=un("dlncm"))
        layernorm(b, x7_m, 1, deccm)
        pse = psmm.tile([NOUT, 512], FP32, tag="mm", name=un("pse"))
        for ct in range(4):
            nc.tensor.matmul(pse[:], projw[:, ct * NOUT:(ct + 1) * NOUT],
                             deccm[:, ct * L + 512: ct * L + 1024],
                             start=(ct == 0), stop=(ct == 3))
        tr16 = scp.tile([NOUT, 512], FP16, tag="tr16", bufs=1, name=un("tr16"))
        nc.vector.tensor_copy(tr16[:], trsb[:, 512:L])
        ptr2 = psmm.tile([NOUT, 512], FP32, tag="mm", name=un("ptr2"))
        nc.tensor.matmul(ptr2[:], tprojw[:], tr16[:], start=True, stop=True)
        seas_sb = scp.tile([NOUT, 512], FP32, tag="seas_sb", bufs=1, name=un("sesb"))
        nc.scalar.copy(seas_sb[:], pse[:])
        fin = scp.tile([NOUT, 512], FP32, tag="fin", bufs=1, name=un("fin"))
        nc.vector.tensor_add(fin[:], seas_sb[:], ptr2[:])
        nc.sync.dma_start(OUTT[b, :, :], fin[:])

    ctx.close()


def _prep_inputs(x_enc, params):
    tens = _flat_params(params)
    x = np.asarray(x_enc, np.float32)
    in_maps = []
    for c in range(NCORES):
        xs = x[c * BLOC:(c + 1) * BLOC]
        m = dict(tens)
        m['x_tm32'] = np.ascontiguousarray(xs)
        m['x_tm16'] = _f16(xs)
        cm = np.transpose(xs, (0, 2, 1))
        cmw = np.concatenate([cm[:, :, -1:], cm, cm[:, :, :1]], axis=2)
        m['x_cm16'] = _f16(cmw)
        in_maps.append(m)
    return in_maps


def kernel(x_enc, params):
    from concourse import bass_utils
    if 'nc' not in _CACHE:
        _CACHE['nc'] = _build_program()
    nc = _CACHE['nc']
    in_maps = _prep_inputs(x_enc, params)
    res = bass_utils.run_bass_kernel_spmd(nc, in_maps, core_ids=list(range(NCORES)))
    _CACHE['last_res'] = res
    outs = []
    for c in range(NCORES):
        o = res.results[c]['out']
        outs.append(np.transpose(o, (0, 2, 1)))
    return np.concatenate(outs, axis=0).astype(np.float32)
